# revision 24
# baseline (speedup 1.0000x reference)
"""Fused multi-head attention (qkv + RoPE + softmax + proj) for TRN2, 8 cores.

Sharding: core c -> batch b=c//2, head group hg=c%2 (8 of 16 heads).
Data-parallel over B (4), 2-way tensor-parallel over heads.
Host unshard: out[b] = partial[2b] + partial[2b+1] + b_proj.

v3: fully phase-overlapped schedule, bf16 storage.
  The scalar engine's exp stream (256 x ~1.1us) is the critical chain:
    1. Head: per n-chunk, emit v units + k(hp0) unit, interleaved with the
       4 attention ki-steps of (hp0, qc0) they unblock -> first exp ~15us.
    2. q/k units for later head pairs are PE "filler" paced inside the
       exp-bound attention loop; proj units likewise during hp3.
    3. Scalar engine executes nothing but exp; gpsimd handles ONLY
       partition_broadcast (mixing op types forces ~7us Q7 library
       reloads); all copies/muls/adds are on vector.
    4. o_ps psum release is decoupled from the softmax normalize: one
       copy to an f32 staging tile frees the bank; recip/broadcast/mul
       into aoT run off the critical path.
  qT/kT/aoT are split into per-chunk tiles so whole-tile dependency
  tracking doesn't serialize the overlap. Storage is bf16 (same PE
  speed, half the SBUF/DMA); rope arithmetic stays f32 (numpy-simulated
  rel err 8e-3 vs the 2e-2 gate).
"""

import sys

if "/opt/trn_rl_repo" not in sys.path:
    sys.path.insert(0, "/opt/trn_rl_repo")

import numpy as np
from contextlib import ExitStack

B, N, C, H, D = 4, 2048, 1024, 16, 64
NCORES = 8
P = 128
DH = 512          # per-core head channels (8 heads x 64)
CT = C // P       # 8 contraction tiles for qkv
DHT = DH // P     # 4 partition tiles of qT/kT/aoT (= head pairs)
NT = N // P       # 16 n tiles
NCH = N // 512    # 4 n chunks of 512
KT = N // P       # 16 key tiles

_CACHE = {}


def _emit(nc, tc, mybir, bass, tile):
    F32 = mybir.dt.float32
    BF16 = mybir.dt.bfloat16
    Exp = mybir.ActivationFunctionType.Exp

    xT = nc.dram_tensor("xT", [C, N], BF16, kind="ExternalInput").ap()
    wq = nc.dram_tensor("wq", [C, DH], BF16, kind="ExternalInput").ap()
    wk = nc.dram_tensor("wk", [C, DH], BF16, kind="ExternalInput").ap()
    wv = nc.dram_tensor("wv", [C, DH], BF16, kind="ExternalInput").ap()
    wp = nc.dram_tensor("wp", [DH, C], BF16, kind="ExternalInput").ap()
    cos2 = nc.dram_tensor("cos2", [P, N], F32, kind="ExternalInput").ap()
    sin2 = nc.dram_tensor("sin2", [P, N], F32, kind="ExternalInput").ap()
    p2t = nc.dram_tensor("p2t", [P, P], BF16, kind="ExternalInput").ap()
    onesd = nc.dram_tensor("onesd", [P, 8], BF16, kind="ExternalInput").ap()
    out = nc.dram_tensor("out", [N, C], BF16, kind="ExternalOutput").ap()

    ctx = ExitStack()
    with ctx:
        consts = ctx.enter_context(tc.tile_pool(name="consts", bufs=1))
        persist = ctx.enter_context(tc.tile_pool(name="persist", bufs=1))

        cos_c = [consts.tile([P, 512], F32, tag=f"cos{c}", name=f"cos{c}")
                 for c in range(NCH)]
        sin_c = [consts.tile([P, 512], F32, tag=f"sin{c}", name=f"sin{c}")
                 for c in range(NCH)]
        p2t_sb = consts.tile([P, P], BF16, tag="p2t")

        # per-chunk tiles -> fine-grained dependencies for the overlap
        qTc = [[persist.tile([P, 512], BF16, tag=f"qT{t}_{c}",
                             name=f"qT{t}_{c}")
                for c in range(NCH)] for t in range(DHT)]
        kTc = [[persist.tile([P, 512], BF16, tag=f"kT{t}_{c}",
                             name=f"kT{t}_{c}")
                for c in range(NCH)] for t in range(DHT)]
        aoTc = [[persist.tile([P, 512], BF16, tag=f"aoT{t}_{c}",
                              name=f"aoT{t}_{c}")
                 for c in range(NCH)] for t in range(DHT)]
        v_sb = [persist.tile([P, 8 * 72], BF16, tag=f"v{i}", name=f"v{i}")
                for i in range(NT)]
        wp_sb = [persist.tile([P, C], BF16, tag=f"wp{i}", name=f"wp{i}")
                 for i in range(DHT)]

        wpool = ctx.enter_context(tc.tile_pool(name="wqkv", bufs=1))
        xpool = ctx.enter_context(tc.tile_pool(name="xt", bufs=1))
        tpool = ctx.enter_context(tc.tile_pool(name="p1tmp", bufs=2))
        epool = ctx.enter_context(tc.tile_pool(name="epool", bufs=6))
        atmp = ctx.enter_context(tc.tile_pool(name="atmp", bufs=2))

        # per-(kc, head-pair) weight tiles: head-pair 0 can DMA first
        wq_sb = [[wpool.tile([P, P], BF16, tag=f"wq{i}_{t}",
                             name=f"wq{i}_{t}") for t in range(DHT)]
                 for i in range(CT)]
        wk_sb = [[wpool.tile([P, P], BF16, tag=f"wk{i}_{t}",
                             name=f"wk{i}_{t}") for t in range(DHT)]
                 for i in range(CT)]
        wv_sb = [wpool.tile([P, DH], BF16, tag=f"wv{i}", name=f"wv{i}")
                 for i in range(CT)]
        xs = [[xpool.tile([P, 512], BF16, tag=f"x{c}_{k}", name=f"x{c}_{k}")
               for k in range(CT)] for c in range(NCH)]

        # ---- DMA issue order: the first scores' deps come first ----
        for kc in range(CT):
            sl = slice(kc * P, (kc + 1) * P)
            nc.sync.dma_start(xs[0][kc], xT[kc * P:(kc + 1) * P, 0:512])
            nc.sync.dma_start(wk_sb[kc][0], wk[sl, 0:P])
        for kc in range(CT):
            nc.sync.dma_start(wq_sb[kc][0], wq[kc * P:(kc + 1) * P, 0:P])
        nc.sync.dma_start(cos_c[0], cos2[:, 0:512])
        nc.sync.dma_start(sin_c[0], sin2[:, 0:512])
        nc.sync.dma_start(p2t_sb, p2t)
        for kc in range(CT):
            nc.sync.dma_start(wv_sb[kc], wv[kc * P:(kc + 1) * P, :])
        def dma_ones(i):
            ones_cols = bass.AP(
                tensor=v_sb[i].tensor, offset=64,
                ap=[list(v_sb[i].ap[0]), [72, 8]])
            nc.sync.dma_start(ones_cols, onesd)
        for i in range(4):
            dma_ones(i)
        for c in range(1, NCH):
            nsl = slice(c * 512, (c + 1) * 512)
            for kc in range(CT):
                nc.sync.dma_start(xs[c][kc], xT[kc * P:(kc + 1) * P, nsl])
            nc.sync.dma_start(cos_c[c], cos2[:, nsl])
            nc.sync.dma_start(sin_c[c], sin2[:, nsl])
            for i in range(c * 4, c * 4 + 4):
                dma_ones(i)
        for t in range(1, DHT):
            tsl = slice(t * P, (t + 1) * P)
            for kc in range(CT):
                sl = slice(kc * P, (kc + 1) * P)
                nc.sync.dma_start(wk_sb[kc][t], wk[sl, tsl])
                nc.sync.dma_start(wq_sb[kc][t], wq[sl, tsl])
        for i in range(DHT):
            nc.sync.dma_start(wp_sb[i], wp[i * P:(i + 1) * P, :])

        # ---------------- unit emitters ----------------
        def make_v_unit(pool, c, nt4):
            def emit():
                i = c * 4 + nt4
                ps = pool.tile([P, 512], F32, tag="vq")
                for kc in range(CT):
                    nc.tensor.matmul(
                        ps, xs[c][kc][:, nt4 * P:(nt4 + 1) * P], wv_sb[kc],
                        start=(kc == 0), stop=(kc == CT - 1))
                v_view = bass.AP(
                    tensor=v_sb[i].tensor, offset=0,
                    ap=[list(v_sb[i].ap[0]), [72, 8], [1, 64]])
                nc.vector.tensor_copy(v_view, ps.rearrange(
                    "p (h d) -> p h d", h=8))
            return emit

        def make_qk_unit(pool, rot_pool, w_sb, dst, t, c):
            def emit():
                ps = pool.tile([P, 512], F32, tag="vq")
                for kc in range(CT):
                    nc.tensor.matmul(
                        ps, w_sb[kc][t], xs[c][kc],
                        start=(kc == 0), stop=(kc == CT - 1))
                raw = tpool.tile([P, 512], BF16, tag="raw")
                nc.vector.tensor_copy(raw, ps)
                rot = rot_pool.tile([P, 512], F32, tag="rot_ps")
                nc.tensor.matmul(rot, p2t_sb, raw, start=True, stop=True)
                t1 = tpool.tile([P, 512], F32, tag="t1")
                nc.vector.tensor_mul(t1, ps, cos_c[c])
                t2 = tpool.tile([P, 512], F32, tag="t2")
                nc.vector.tensor_mul(t2, rot, sin_c[c])
                nc.vector.tensor_add(dst[t][c], t1, t2)
            return emit

        proj_pool_holder = [None]

        def make_proj_emitter(nt, fc):
            def emit():
                ps = proj_pool_holder[0].tile([P, 512], F32, tag="pps")
                for ct in range(DHT):
                    nc.tensor.matmul(
                        ps,
                        aoTc[ct][nt // 4][:, (nt % 4) * P:(nt % 4 + 1) * P],
                        wp_sb[ct][:, fc * 512:(fc + 1) * 512],
                        start=(ct == 0), stop=(ct == DHT - 1))
                ob = atmp.tile([P, 512], BF16, tag="ob")
                nc.vector.tensor_copy(ob, ps)
                nc.sync.dma_start(
                    out[nt * P:(nt + 1) * P, fc * 512:(fc + 1) * 512], ob)
            return emit

        # persistent attention pools
        o_ps_pool = ctx.enter_context(
            tc.tile_pool(name="o_ps", bufs=1, space="PSUM"))

        def attn_ki(hp, qc, ki, s_pool, state):
            """one ki step of attention block (hp, qc)"""
            o_ps = state["o"]
            ksl = slice((ki % 4) * P, (ki % 4 + 1) * P)
            s_ps = s_pool.tile([P, 2, 512], F32, tag=state["stag"](ki))
            for par in range(2):
                pb = par * 64
                nc.tensor.matmul(
                    s_ps[:, par],
                    kTc[hp][ki // 4][pb:pb + 64, ksl],
                    qTc[hp][qc][pb:pb + 64, :],
                    start=True, stop=True,
                    tile_position=(pb, 0))
            if len(state["pending"]) >= 4:
                emit_av(hp, o_ps, *state["pending"].pop(0))
            e = epool.tile([P, 2, 512], BF16, tag="e", name="e")
            nc.scalar.activation(e, s_ps, Exp, scale=float(D) ** -0.5)
            state["pending"].append((ki, e))

        def emit_av(hp, o_ps, ki, e):
            for par in range(2):
                h = hp * 2 + par
                nc.tensor.matmul(
                    o_ps[par][0:65, :],
                    v_sb[ki][:, h * 72:h * 72 + 65],
                    e[:, par],
                    start=(ki == 0), stop=(ki == KT - 1))

        def finish_block(hp, qc, state):
            o_ps = state["o"]
            for item in state["pending"]:
                emit_av(hp, o_ps, *item)
            for par in range(2):
                pb = par * 64
                stage = atmp.tile([P, 512], F32, tag="stg")
                nc.vector.tensor_copy(stage[0:64, :], o_ps[par][0:64, :])
                dd = atmp.tile([P, 512], F32, tag="dd")
                nc.vector.tensor_copy(dd[0:1, :], o_ps[par][64:65, :])
                r = atmp.tile([P, 512], F32, tag="r")
                nc.vector.reciprocal_approx_fast(r[0:1, :], dd[0:1, :])
                rb = atmp.tile([P, 512], F32, tag="rb")
                nc.gpsimd.partition_broadcast(
                    rb[0:64, :], r[0:1, :], channels=64)
                nc.vector.tensor_mul(
                    aoTc[hp][qc][pb:pb + 64, :],
                    stage[0:64, :],
                    rb[0:64, :])

        def new_block_state(stag):
            o_ps = {par: o_ps_pool.tile([P, 512], F32, tag=f"o{par}",
                                        name=f"o{par}") for par in range(2)}
            return {"o": o_ps, "pending": [], "stag": stag}

        # ---------------- head: chunk-interleaved (hp0, qc0) ----------------
        head_ctx = ExitStack()
        hpool = head_ctx.enter_context(
            tc.tile_pool(name="hvq", bufs=2, space="PSUM"))
        hrot = head_ctx.enter_context(
            tc.tile_pool(name="hrot", bufs=1, space="PSUM"))
        s_head = head_ctx.enter_context(
            tc.tile_pool(name="s_head", bufs=1, space="PSUM"))

        state = new_block_state(lambda ki: "sH")
        for c in range(NCH):
            make_qk_unit(hpool, hrot, wk_sb, kTc, 0, c)()
            if c == 0:
                make_qk_unit(hpool, hrot, wq_sb, qTc, 0, 0)()
            for ki in range(c * 4, c * 4 + 4):
                attn_ki(0, 0, ki, s_head, state)
            for nt4 in range(4):
                make_v_unit(hpool, c, nt4)()
            if c == NCH - 1:
                # qc1's q unit one block early, off its critical path
                make_qk_unit(hpool, hrot, wq_sb, qTc, 0, 1)()
        finish_block(0, 0, state)
        head_ctx.close()

        # ---------------- main: attention + paced fillers ----------------
        s_main = ctx.enter_context(
            tc.tile_pool(name="s_ps", bufs=1, space="PSUM"))
        fqk_ctx = ExitStack()
        fqk = fqk_ctx.enter_context(
            tc.tile_pool(name="fqk", bufs=1, space="PSUM"))
        frot = fqk_ctx.enter_context(
            tc.tile_pool(name="frot", bufs=1, space="PSUM"))

        # paced filler queue per hp = q/k units of the next head pair,
        # k-first (next hp's first scores need k chunk0 + q qc0 earliest)
        fillers = []
        for t in range(1, DHT):
            unit_list = []
            for c in range(NCH):
                unit_list.append(make_qk_unit(fqk, frot, wk_sb, kTc, t, c))
                unit_list.append(make_qk_unit(fqk, frot, wq_sb, qTc, t, c))
            fillers.append(unit_list)
        pending_proj = []
        hoisted = None

        for hp in range(DHT):
            if hp == DHT - 1:
                fqk_ctx.close()
                proj_pool_holder[0] = ctx.enter_context(
                    tc.tile_pool(name="proj_ps", bufs=2, space="PSUM"))
            queue = fillers[hp] if hp < DHT - 1 else None
            for qc in range(NCH):
                if hp == 0 and qc == 0:
                    continue  # done in the head
                if hoisted is not None and hoisted[0] == (hp, qc):
                    state = hoisted[1]
                    start_ki = 1
                else:
                    state = new_block_state(lambda ki: f"s{ki % 2}")
                    start_ki = 0
                hoisted = None
                for ki in range(start_ki, KT):
                    attn_ki(hp, qc, ki, s_main, state)
                    if hp == 0 and qc < NCH - 1 and ki == 3:
                        # next block's q unit, one block early
                        make_qk_unit(fqk, frot, wq_sb, qTc, 0, qc + 1)()
                    if queue is not None:
                        if ki % 2 == 1:
                            slot = (qc * KT + ki) // 2
                            if slot % 3 != 2 and queue:
                                queue.pop(0)()
                    elif ki % 2 == 1 and pending_proj:
                        pending_proj.pop(0)()
                if queue is not None and qc == NCH - 1:
                    while queue:
                        queue.pop(0)()
                # software-pipeline: next block's first scores+exp go ahead
                # of this block's AV drain + normalize so ACT never gaps
                nxt = (hp, qc + 1) if qc + 1 < NCH else (hp + 1, 0)
                if nxt[0] < DHT:
                    nstate = new_block_state(lambda ki: f"s{ki % 2}")
                    attn_ki(nxt[0], nxt[1], 0, s_main, nstate)
                    hoisted = (nxt, nstate)
                finish_block(hp, qc, state)
                if hp == DHT - 1 and qc < NCH - 1:
                    for nt in range(qc * 4, (qc + 1) * 4):
                        for fc in range(2):
                            pending_proj.append(make_proj_emitter(nt, fc))
        for emit in pending_proj:
            emit()
        # last q-chunk's proj: hp0-2 partial sums run during the final
        # softmax normalize; only the hp3 matmul waits for it
        final_units = [(nt, fc) for nt in range((NCH - 1) * 4, NCH * 4)
                       for fc in range(2)]
        for i in range(0, len(final_units), 2):
            pair = final_units[i:i + 2]
            pss = []
            for nt, fc in pair:
                ps = proj_pool_holder[0].tile([P, 512], F32, tag="pps",
                                              name="pps")
                for ct in range(DHT - 1):
                    nc.tensor.matmul(
                        ps,
                        aoTc[ct][nt // 4][:, (nt % 4) * P:(nt % 4 + 1) * P],
                        wp_sb[ct][:, fc * 512:(fc + 1) * 512],
                        start=(ct == 0), stop=False)
                pss.append(ps)
            for (nt, fc), ps in zip(pair, pss):
                ct = DHT - 1
                nc.tensor.matmul(
                    ps,
                    aoTc[ct][nt // 4][:, (nt % 4) * P:(nt % 4 + 1) * P],
                    wp_sb[ct][:, fc * 512:(fc + 1) * 512],
                    start=False, stop=True)
                ob = atmp.tile([P, 512], BF16, tag="ob")
                nc.vector.tensor_copy(ob, ps)
                nc.sync.dma_start(
                    out[nt * P:(nt + 1) * P, fc * 512:(fc + 1) * 512], ob)


def build_nc():
    if "nc" in _CACHE:
        return _CACHE["nc"]
    import concourse.bass as bass
    import concourse.tile as tile
    from concourse import bacc, mybir

    nc = bacc.Bacc("TRN2", target_bir_lowering=False, debug=False,
                   enable_asserts=False, num_devices=NCORES)
    with tile.TileContext(nc) as tc:
        _emit(nc, tc, mybir, bass, tile)
    nc.compile()
    _CACHE["nc"] = nc
    return nc


def make_in_maps(x, rope_cos, rope_sin, w_qkv, w_proj):
    import ml_dtypes
    BF = ml_dtypes.bfloat16
    x = np.asarray(x, dtype=np.float32)
    rope_cos = np.asarray(rope_cos, dtype=np.float32)
    rope_sin = np.asarray(rope_sin, dtype=np.float32)
    w_qkv = np.asarray(w_qkv, dtype=np.float32)
    w_proj = np.asarray(w_proj, dtype=np.float32)

    cosT = np.ascontiguousarray(rope_cos.T)           # [64, N]
    cos2 = np.vstack([cosT, cosT])                    # [128, N]
    sinT = np.ascontiguousarray(rope_sin.T)
    sin2 = np.vstack([sinT, sinT])

    # signed half-rotation permutation: rot(q) = P2 @ q (per 64-block)
    R = np.zeros((D, D), dtype=np.float32)
    half = D // 2
    R[np.arange(half), np.arange(half) + half] = -1.0
    R[np.arange(half) + half, np.arange(half)] = 1.0
    P2 = np.zeros((P, P), dtype=np.float32)
    P2[:D, :D] = R
    P2[D:, D:] = R
    p2t = np.ascontiguousarray(P2.T).astype(BF)

    xTs = [np.ascontiguousarray(x[b].T).astype(BF) for b in range(B)]

    in_maps = []
    for core in range(NCORES):
        b = core // 2
        hg = core % 2
        in_maps.append({
            "xT": xTs[b],
            "wq": np.ascontiguousarray(
                w_qkv[hg * DH:(hg + 1) * DH, :].T).astype(BF),
            "wk": np.ascontiguousarray(
                w_qkv[C + hg * DH:C + (hg + 1) * DH, :].T).astype(BF),
            "wv": np.ascontiguousarray(
                w_qkv[2 * C + hg * DH:2 * C + (hg + 1) * DH, :].T).astype(BF),
            "wp": np.ascontiguousarray(
                w_proj[:, hg * DH:(hg + 1) * DH].T).astype(BF),
            "cos2": cos2,
            "sin2": sin2,
            "p2t": p2t,
            "onesd": np.ones((P, 8), dtype=BF),
        })
    return in_maps


def kernel(x, rope_cos, rope_sin, w_qkv, w_proj, b_proj, trace=False):
    from concourse.bass_utils import run_bass_kernel_spmd

    nc = build_nc()
    in_maps = make_in_maps(x, rope_cos, rope_sin, w_qkv, w_proj)
    res = run_bass_kernel_spmd(nc, in_maps, core_ids=list(range(NCORES)),
                               trace=trace)
    b_proj = np.asarray(b_proj, dtype=np.float32)
    final = np.empty((B, N, C), dtype=np.float32)
    for b in range(B):
        final[b] = (res.results[2 * b]["out"].astype(np.float32)
                    + res.results[2 * b + 1]["out"].astype(np.float32)
                    + b_proj)
    if trace:
        kernel.last_exec_time_ns = res.exec_time_ns
        kernel.last_results = res
    return final


# revision 25
# speedup vs baseline: 1.1741x; 1.1741x over previous
"""Fused multi-head attention (qkv + RoPE + softmax + proj) for TRN2, 8 cores.

Sharding: core c -> batch b=c//2, head group hg=c%2 (8 of 16 heads).
Data-parallel over B (4), 2-way tensor-parallel over heads.
Host unshard: out[b] = partial[2b] + partial[2b+1] + b_proj.

v3: fully phase-overlapped schedule, bf16 storage.
  The scalar engine's exp stream (256 x ~1.1us) is the critical chain:
    1. Head: per n-chunk, emit v units + k(hp0) unit, interleaved with the
       4 attention ki-steps of (hp0, qc0) they unblock -> first exp ~15us.
    2. q/k units for later head pairs are PE "filler" paced inside the
       exp-bound attention loop; proj units likewise during hp3.
    3. Scalar engine executes nothing but exp; gpsimd handles ONLY
       partition_broadcast (mixing op types forces ~7us Q7 library
       reloads); all copies/muls/adds are on vector.
    4. o_ps psum release is decoupled from the softmax normalize: one
       copy to an f32 staging tile frees the bank; recip/broadcast/mul
       into aoT run off the critical path.
  qT/kT/aoT are split into per-chunk tiles so whole-tile dependency
  tracking doesn't serialize the overlap. Storage is bf16 (same PE
  speed, half the SBUF/DMA); rope arithmetic stays f32 (numpy-simulated
  rel err 8e-3 vs the 2e-2 gate).
"""

import sys

if "/opt/trn_rl_repo" not in sys.path:
    sys.path.insert(0, "/opt/trn_rl_repo")

import numpy as np
from contextlib import ExitStack

B, N, C, H, D = 4, 2048, 1024, 16, 64
NCORES = 8
P = 128
DH = 512          # per-core head channels (8 heads x 64)
CT = C // P       # 8 contraction tiles for qkv
DHT = DH // P     # 4 partition tiles of qT/kT/aoT (= head pairs)
NT = N // P       # 16 n tiles
NCH = N // 512    # 4 n chunks of 512
KT = N // P       # 16 key tiles

_CACHE = {}


def _emit(nc, tc, mybir, bass, tile):
    F32 = mybir.dt.float32
    BF16 = mybir.dt.bfloat16
    Exp = mybir.ActivationFunctionType.Exp

    xT = nc.dram_tensor("xT", [C, N], BF16, kind="ExternalInput").ap()
    wq = nc.dram_tensor("wq", [C, DH], BF16, kind="ExternalInput").ap()
    wk = nc.dram_tensor("wk", [C, DH], BF16, kind="ExternalInput").ap()
    wv = nc.dram_tensor("wv", [C, DH], BF16, kind="ExternalInput").ap()
    wp = nc.dram_tensor("wp", [DH, C], BF16, kind="ExternalInput").ap()
    cos2 = nc.dram_tensor("cos2", [P, N], F32, kind="ExternalInput").ap()
    sin2 = nc.dram_tensor("sin2", [P, N], F32, kind="ExternalInput").ap()
    p2t = nc.dram_tensor("p2t", [P, P], BF16, kind="ExternalInput").ap()
    onesd = nc.dram_tensor("onesd", [P, 8], BF16, kind="ExternalInput").ap()
    out = nc.dram_tensor("out", [N, C], BF16, kind="ExternalOutput").ap()

    ctx = ExitStack()
    with ctx:
        consts = ctx.enter_context(tc.tile_pool(name="consts", bufs=1))
        persist = ctx.enter_context(tc.tile_pool(name="persist", bufs=1))

        cos_c = [consts.tile([P, 512], F32, tag=f"cos{c}", name=f"cos{c}")
                 for c in range(NCH)]
        sin_c = [consts.tile([P, 512], F32, tag=f"sin{c}", name=f"sin{c}")
                 for c in range(NCH)]
        p2t_sb = consts.tile([P, P], BF16, tag="p2t")

        # per-chunk tiles -> fine-grained dependencies for the overlap
        qTc = [[persist.tile([P, 512], BF16, tag=f"qT{t}_{c}",
                             name=f"qT{t}_{c}")
                for c in range(NCH)] for t in range(DHT)]
        kTc = [[persist.tile([P, 512], BF16, tag=f"kT{t}_{c}",
                             name=f"kT{t}_{c}")
                for c in range(NCH)] for t in range(DHT)]
        aoTc = [[persist.tile([P, 512], BF16, tag=f"aoT{t}_{c}",
                              name=f"aoT{t}_{c}")
                 for c in range(NCH)] for t in range(DHT)]
        v_sb = [persist.tile([P, 8 * 72], BF16, tag=f"v{i}", name=f"v{i}")
                for i in range(NT)]
        wp_sb = [persist.tile([P, C], BF16, tag=f"wp{i}", name=f"wp{i}")
                 for i in range(DHT)]

        wpool = ctx.enter_context(tc.tile_pool(name="wqkv", bufs=1))
        xpool = ctx.enter_context(tc.tile_pool(name="xt", bufs=1))
        tpool = ctx.enter_context(tc.tile_pool(name="p1tmp", bufs=2))
        epool = ctx.enter_context(tc.tile_pool(name="epool", bufs=6))
        atmp = ctx.enter_context(tc.tile_pool(name="atmp", bufs=2))

        # per-(kc, head-pair) weight tiles: head-pair 0 can DMA first
        wq_sb = [[wpool.tile([P, P], BF16, tag=f"wq{i}_{t}",
                             name=f"wq{i}_{t}") for t in range(DHT)]
                 for i in range(CT)]
        wk_sb = [[wpool.tile([P, P], BF16, tag=f"wk{i}_{t}",
                             name=f"wk{i}_{t}") for t in range(DHT)]
                 for i in range(CT)]
        wv_sb = [wpool.tile([P, DH], BF16, tag=f"wv{i}", name=f"wv{i}")
                 for i in range(CT)]
        xs = [[xpool.tile([P, 512], BF16, tag=f"x{c}_{k}", name=f"x{c}_{k}")
               for k in range(CT)] for c in range(NCH)]

        # ---- DMA issue order: the first scores' deps come first ----
        for kc in range(CT):
            sl = slice(kc * P, (kc + 1) * P)
            nc.sync.dma_start(xs[0][kc], xT[kc * P:(kc + 1) * P, 0:512])
            nc.sync.dma_start(wk_sb[kc][0], wk[sl, 0:P])
        for kc in range(CT):
            nc.sync.dma_start(wq_sb[kc][0], wq[kc * P:(kc + 1) * P, 0:P])
        nc.sync.dma_start(cos_c[0], cos2[:, 0:512])
        nc.sync.dma_start(sin_c[0], sin2[:, 0:512])
        nc.sync.dma_start(p2t_sb, p2t)
        for kc in range(CT):
            nsl1 = slice(512, 1024)
            nc.sync.dma_start(xs[1][kc], xT[kc * P:(kc + 1) * P, nsl1])
        for kc in range(CT):
            nc.sync.dma_start(wv_sb[kc], wv[kc * P:(kc + 1) * P, :])
        def dma_ones(i):
            ones_cols = bass.AP(
                tensor=v_sb[i].tensor, offset=64,
                ap=[list(v_sb[i].ap[0]), [72, 8]])
            nc.sync.dma_start(ones_cols, onesd)
        for i in range(4):
            dma_ones(i)
        for c in range(1, NCH):
            nsl = slice(c * 512, (c + 1) * 512)
            if c > 1:
                for kc in range(CT):
                    nc.sync.dma_start(xs[c][kc], xT[kc * P:(kc + 1) * P, nsl])
            nc.sync.dma_start(cos_c[c], cos2[:, nsl])
            nc.sync.dma_start(sin_c[c], sin2[:, nsl])
            for i in range(c * 4, c * 4 + 4):
                dma_ones(i)
        for t in range(1, DHT):
            tsl = slice(t * P, (t + 1) * P)
            for kc in range(CT):
                sl = slice(kc * P, (kc + 1) * P)
                nc.sync.dma_start(wk_sb[kc][t], wk[sl, tsl])
                nc.sync.dma_start(wq_sb[kc][t], wq[sl, tsl])
        for i in range(DHT):
            nc.sync.dma_start(wp_sb[i], wp[i * P:(i + 1) * P, :])

        # ---------------- unit emitters ----------------
        def make_v_unit(pool, c, nt4):
            def emit():
                i = c * 4 + nt4
                ps = pool.tile([P, 512], F32, tag="vq")
                for kc in range(CT):
                    nc.tensor.matmul(
                        ps, xs[c][kc][:, nt4 * P:(nt4 + 1) * P], wv_sb[kc],
                        start=(kc == 0), stop=(kc == CT - 1))
                v_view = bass.AP(
                    tensor=v_sb[i].tensor, offset=0,
                    ap=[list(v_sb[i].ap[0]), [72, 8], [1, 64]])
                nc.vector.tensor_copy(v_view, ps.rearrange(
                    "p (h d) -> p h d", h=8))
            return emit

        def make_qk_unit(pool, rot_pool, w_sb, dst, t, c):
            def emit():
                ps = pool.tile([P, 512], F32, tag="vq")
                for kc in range(CT):
                    nc.tensor.matmul(
                        ps, w_sb[kc][t], xs[c][kc],
                        start=(kc == 0), stop=(kc == CT - 1))
                raw = tpool.tile([P, 512], BF16, tag="raw")
                nc.vector.tensor_copy(raw, ps)
                rot = rot_pool.tile([P, 512], F32, tag="rot_ps")
                nc.tensor.matmul(rot, p2t_sb, raw, start=True, stop=True)
                t1 = tpool.tile([P, 512], F32, tag="t1")
                nc.vector.tensor_mul(t1, ps, cos_c[c])
                t2 = tpool.tile([P, 512], F32, tag="t2")
                nc.vector.tensor_mul(t2, rot, sin_c[c])
                nc.vector.tensor_add(dst[t][c], t1, t2)
            return emit

        proj_pool_holder = [None]

        def make_proj_emitter(nt, fc):
            def emit():
                ps = proj_pool_holder[0].tile([P, 512], F32, tag="pps")
                for ct in range(DHT):
                    nc.tensor.matmul(
                        ps,
                        aoTc[ct][nt // 4][:, (nt % 4) * P:(nt % 4 + 1) * P],
                        wp_sb[ct][:, fc * 512:(fc + 1) * 512],
                        start=(ct == 0), stop=(ct == DHT - 1))
                ob = atmp.tile([P, 512], BF16, tag="ob")
                nc.vector.tensor_copy(ob, ps)
                nc.sync.dma_start(
                    out[nt * P:(nt + 1) * P, fc * 512:(fc + 1) * 512], ob)
            return emit

        # persistent attention pools
        o_ps_pool = ctx.enter_context(
            tc.tile_pool(name="o_ps", bufs=1, space="PSUM"))

        def attn_ki(hp, qc, ki, s_pool, state):
            """one ki step of attention block (hp, qc)"""
            o_ps = state["o"]
            ksl = slice((ki % 4) * P, (ki % 4 + 1) * P)
            s_ps = s_pool.tile([P, 2, 512], F32, tag=state["stag"](ki))
            for par in range(2):
                pb = par * 64
                nc.tensor.matmul(
                    s_ps[:, par],
                    kTc[hp][ki // 4][pb:pb + 64, ksl],
                    qTc[hp][qc][pb:pb + 64, :],
                    start=True, stop=True,
                    tile_position=(pb, 0))
            if len(state["pending"]) >= 4:
                emit_av(hp, o_ps, *state["pending"].pop(0))
            e = epool.tile([P, 2, 512], BF16, tag="e", name="e")
            nc.scalar.activation(e, s_ps, Exp, scale=float(D) ** -0.5)
            state["pending"].append((ki, e))

        def emit_av(hp, o_ps, ki, e):
            for par in range(2):
                h = hp * 2 + par
                nc.tensor.matmul(
                    o_ps[par][0:65, :],
                    v_sb[ki][:, h * 72:h * 72 + 65],
                    e[:, par],
                    start=(ki == 0), stop=(ki == KT - 1))

        def finish_block(hp, qc, state):
            o_ps = state["o"]
            for item in state["pending"]:
                emit_av(hp, o_ps, *item)
            for par in range(2):
                pb = par * 64
                stage = atmp.tile([P, 512], F32, tag="stg")
                nc.vector.tensor_copy(stage[0:64, :], o_ps[par][0:64, :])
                dd = atmp.tile([P, 512], F32, tag="dd")
                nc.vector.tensor_copy(dd[0:1, :], o_ps[par][64:65, :])
                r = atmp.tile([P, 512], F32, tag="r")
                nc.vector.reciprocal_approx_fast(r[0:1, :], dd[0:1, :])
                rb = atmp.tile([P, 512], F32, tag="rb")
                nc.gpsimd.partition_broadcast(
                    rb[0:64, :], r[0:1, :], channels=64)
                nc.vector.tensor_mul(
                    aoTc[hp][qc][pb:pb + 64, :],
                    stage[0:64, :],
                    rb[0:64, :])

        def new_block_state(stag):
            o_ps = {par: o_ps_pool.tile([P, 512], F32, tag=f"o{par}",
                                        name=f"o{par}") for par in range(2)}
            return {"o": o_ps, "pending": [], "stag": stag}

        # ---------------- head: chunk-interleaved (hp0, qc0) ----------------
        head_ctx = ExitStack()
        hpool = head_ctx.enter_context(
            tc.tile_pool(name="hvq", bufs=2, space="PSUM"))
        hrot = head_ctx.enter_context(
            tc.tile_pool(name="hrot", bufs=1, space="PSUM"))
        s_head = head_ctx.enter_context(
            tc.tile_pool(name="s_head", bufs=1, space="PSUM"))

        state = new_block_state(lambda ki: "sH")
        for c in range(NCH):
            make_qk_unit(hpool, hrot, wk_sb, kTc, 0, c)()
            if c == 0:
                make_qk_unit(hpool, hrot, wq_sb, qTc, 0, 0)()
            else:
                for nt4 in range(4):
                    make_v_unit(hpool, c - 1, nt4)()
            for ki in range(c * 4, c * 4 + 4):
                attn_ki(0, 0, ki, s_head, state)
            if c == NCH - 1:
                for nt4 in range(4):
                    make_v_unit(hpool, c, nt4)()
                # qc1's q unit one block early, off its critical path
                make_qk_unit(hpool, hrot, wq_sb, qTc, 0, 1)()
        finish_block(0, 0, state)
        head_ctx.close()

        # ---------------- main: attention + paced fillers ----------------
        s_main = ctx.enter_context(
            tc.tile_pool(name="s_ps", bufs=1, space="PSUM"))
        fqk_ctx = ExitStack()
        fqk = fqk_ctx.enter_context(
            tc.tile_pool(name="fqk", bufs=1, space="PSUM"))
        frot = fqk_ctx.enter_context(
            tc.tile_pool(name="frot", bufs=1, space="PSUM"))

        # paced filler queue per hp = q/k units of the next head pair,
        # k-first (next hp's first scores need k chunk0 + q qc0 earliest)
        fillers = []
        for t in range(1, DHT):
            unit_list = []
            for c in range(NCH):
                unit_list.append(make_qk_unit(fqk, frot, wk_sb, kTc, t, c))
                unit_list.append(make_qk_unit(fqk, frot, wq_sb, qTc, t, c))
            fillers.append(unit_list)
        pending_proj = []
        hoisted = None

        for hp in range(DHT):
            if hp == DHT - 1:
                fqk_ctx.close()
                proj_pool_holder[0] = ctx.enter_context(
                    tc.tile_pool(name="proj_ps", bufs=2, space="PSUM"))
            queue = fillers[hp] if hp < DHT - 1 else None
            for qc in range(NCH):
                if hp == 0 and qc == 0:
                    continue  # done in the head
                if hoisted is not None and hoisted[0] == (hp, qc):
                    state = hoisted[1]
                    start_ki = 1
                else:
                    state = new_block_state(lambda ki: f"s{ki % 2}")
                    start_ki = 0
                hoisted = None
                for ki in range(start_ki, KT):
                    attn_ki(hp, qc, ki, s_main, state)
                    if hp == 0 and qc < NCH - 1 and ki == 3:
                        # next block's q unit, one block early
                        make_qk_unit(fqk, frot, wq_sb, qTc, 0, qc + 1)()
                    if queue is not None:
                        if ki % 2 == 1:
                            slot = (qc * KT + ki) // 2
                            if slot % 3 != 2 and queue:
                                queue.pop(0)()
                    elif ki % 2 == 1 and pending_proj:
                        pending_proj.pop(0)()
                if queue is not None and qc == NCH - 1:
                    while queue:
                        queue.pop(0)()
                # software-pipeline: next block's first scores+exp go ahead
                # of this block's AV drain + normalize so ACT never gaps
                nxt = (hp, qc + 1) if qc + 1 < NCH else (hp + 1, 0)
                if nxt[0] < DHT:
                    nstate = new_block_state(lambda ki: f"s{ki % 2}")
                    attn_ki(nxt[0], nxt[1], 0, s_main, nstate)
                    hoisted = (nxt, nstate)
                finish_block(hp, qc, state)
                if hp == DHT - 1 and qc < NCH - 1:
                    for nt in range(qc * 4, (qc + 1) * 4):
                        for fc in range(2):
                            pending_proj.append(make_proj_emitter(nt, fc))
        for emit in pending_proj:
            emit()
        # last q-chunk's proj: hp0-2 partial sums run during the final
        # softmax normalize; only the hp3 matmul waits for it
        final_units = [(nt, fc) for nt in range((NCH - 1) * 4, NCH * 4)
                       for fc in range(2)]
        for i in range(0, len(final_units), 2):
            pair = final_units[i:i + 2]
            pss = []
            for nt, fc in pair:
                ps = proj_pool_holder[0].tile([P, 512], F32, tag="pps",
                                              name="pps")
                for ct in range(DHT - 1):
                    nc.tensor.matmul(
                        ps,
                        aoTc[ct][nt // 4][:, (nt % 4) * P:(nt % 4 + 1) * P],
                        wp_sb[ct][:, fc * 512:(fc + 1) * 512],
                        start=(ct == 0), stop=False)
                pss.append(ps)
            for (nt, fc), ps in zip(pair, pss):
                ct = DHT - 1
                nc.tensor.matmul(
                    ps,
                    aoTc[ct][nt // 4][:, (nt % 4) * P:(nt % 4 + 1) * P],
                    wp_sb[ct][:, fc * 512:(fc + 1) * 512],
                    start=False, stop=True)
                ob = atmp.tile([P, 512], BF16, tag="ob")
                nc.vector.tensor_copy(ob, ps)
                nc.sync.dma_start(
                    out[nt * P:(nt + 1) * P, fc * 512:(fc + 1) * 512], ob)


def build_nc():
    if "nc" in _CACHE:
        return _CACHE["nc"]
    import concourse.bass as bass
    import concourse.tile as tile
    from concourse import bacc, mybir

    nc = bacc.Bacc("TRN2", target_bir_lowering=False, debug=False,
                   enable_asserts=False, num_devices=NCORES)
    with tile.TileContext(nc) as tc:
        _emit(nc, tc, mybir, bass, tile)
    nc.compile()
    _CACHE["nc"] = nc
    return nc


def make_in_maps(x, rope_cos, rope_sin, w_qkv, w_proj):
    import ml_dtypes
    BF = ml_dtypes.bfloat16
    x = np.asarray(x, dtype=np.float32)
    rope_cos = np.asarray(rope_cos, dtype=np.float32)
    rope_sin = np.asarray(rope_sin, dtype=np.float32)
    w_qkv = np.asarray(w_qkv, dtype=np.float32)
    w_proj = np.asarray(w_proj, dtype=np.float32)

    cosT = np.ascontiguousarray(rope_cos.T)           # [64, N]
    cos2 = np.vstack([cosT, cosT])                    # [128, N]
    sinT = np.ascontiguousarray(rope_sin.T)
    sin2 = np.vstack([sinT, sinT])

    # signed half-rotation permutation: rot(q) = P2 @ q (per 64-block)
    R = np.zeros((D, D), dtype=np.float32)
    half = D // 2
    R[np.arange(half), np.arange(half) + half] = -1.0
    R[np.arange(half) + half, np.arange(half)] = 1.0
    P2 = np.zeros((P, P), dtype=np.float32)
    P2[:D, :D] = R
    P2[D:, D:] = R
    p2t = np.ascontiguousarray(P2.T).astype(BF)

    xTs = [np.ascontiguousarray(x[b].T).astype(BF) for b in range(B)]

    in_maps = []
    for core in range(NCORES):
        b = core // 2
        hg = core % 2
        in_maps.append({
            "xT": xTs[b],
            "wq": np.ascontiguousarray(
                w_qkv[hg * DH:(hg + 1) * DH, :].T).astype(BF),
            "wk": np.ascontiguousarray(
                w_qkv[C + hg * DH:C + (hg + 1) * DH, :].T).astype(BF),
            "wv": np.ascontiguousarray(
                w_qkv[2 * C + hg * DH:2 * C + (hg + 1) * DH, :].T).astype(BF),
            "wp": np.ascontiguousarray(
                w_proj[:, hg * DH:(hg + 1) * DH].T).astype(BF),
            "cos2": cos2,
            "sin2": sin2,
            "p2t": p2t,
            "onesd": np.ones((P, 8), dtype=BF),
        })
    return in_maps


def kernel(x, rope_cos, rope_sin, w_qkv, w_proj, b_proj, trace=False):
    from concourse.bass_utils import run_bass_kernel_spmd

    nc = build_nc()
    in_maps = make_in_maps(x, rope_cos, rope_sin, w_qkv, w_proj)
    res = run_bass_kernel_spmd(nc, in_maps, core_ids=list(range(NCORES)),
                               trace=trace)
    b_proj = np.asarray(b_proj, dtype=np.float32)
    final = np.empty((B, N, C), dtype=np.float32)
    for b in range(B):
        final[b] = (res.results[2 * b]["out"].astype(np.float32)
                    + res.results[2 * b + 1]["out"].astype(np.float32)
                    + b_proj)
    if trace:
        kernel.last_exec_time_ns = res.exec_time_ns
        kernel.last_results = res
    return final
